# revision 20
# baseline (speedup 1.0000x reference)
"""Trainium2 Bass kernel for DPL safe-policy head.

Computes, for x:[B,H] and three tiny heads Wg/Wp/Wa (4/4/5 logits):
    ghost  = softmax(x@Wg + bg); pacman = softmax(x@Wp + bp); base = softmax(x@Wa + ba)
    unsafe[b,a] = sum_cd pacman[b,c] * T[a,c,d] * ghost[b,d]   (T fixed 0/1 tensor)
    out = base*(1-unsafe) / sum(...)

Closed form used on device (softmax normalizations cancel except ghost/pacman's,
which fold into Sp*Sg):
    E = exp(logits), Sg = sum(EG), Sp = sum(EP), SS = Sp*Sg
    u0 = sum_c EPc*EGc ; u1 = EP0*EG1+EP2*EG3 ; u2 = EP1*EG0+EP3*EG2
    t_j = EA_j * (SS - u_j)  (u3 = u4 = 0);  out_j = t_j / sum_j t_j

Sharding: pure data parallel over batch across 8 cores (2048 rows each).

Per core pipeline (memory-bound target: stream x once from HBM at ~330GB/s;
PE issue rate is the co-bottleneck, so every engine has exactly one job):
  - x streams through the sync HWDGE queue as half-tile [128, 1024] DMAs
    (4 KiB lines); w (host-pretransposed, contiguous), bias and identity
    ride the scalar queue so they are resident in the first ~2us
  - ACT converts each half-tile to fp16 (one pass over x) + 2 exps
  - PE: per 128x128 chunk, one fp16 transpose (1 cy/row) and one fp16
    accumulation matmul (N=13); bias is folded in by the DVE instead of a
    rank-1 matmul; matmul emission lags transposes by one group so the
    in-order PE stream never waits on the DVE copy
  - DVE copies PSUM->SBUF fp16 transposed operands + per-tile bias-add fold
    (reads PSUM, which gpsimd cannot) + 2 reciprocals
  - GpSimd runs the whole logic-layer tail (SBUF-only tensor ops) and the
    output DMA on its software queue, so the DVE/ACT streams never pause
  - output written as one contiguous [128, NT*5] block per half (160B
    partition lines); host reorders to [B, 5]

fp16 single-term matmul (f16x1): max rel err ~1.5e-3 vs the fp32 reference
(harness gate 2e-2).
"""

import numpy as np

import concourse.bass as bass
import concourse.bacc as bacc
import concourse.mybir as mybir
import concourse.tile as tile
from concourse.bass_utils import run_bass_kernel_spmd

F32 = mybir.dt.float32
F16 = mybir.dt.float16
AX = mybir.AxisListType
ADD = mybir.AluOpType.add
SUB = mybir.AluOpType.subtract

MODE = "f16pre"

N_CORES = 8
B_FULL, H = 16384, 2048
B = B_FULL // N_CORES  # rows per core
P = 128
NT = B // P            # batch tiles per core
NCH = H // P           # contraction chunks
GC = 4                 # chunks per psum transpose group
NG = NCH // GC
J = 13                 # 4 + 4 + 5 logits


def _build_program(mode):
    assert mode == "f16pre"
    nc = bacc.Bacc("TRN2", target_bir_lowering=False, debug=False,
                   num_devices=N_CORES)
    x_d = nc.dram_tensor("x", [B, H], F32, kind="ExternalInput")
    w_d = nc.dram_tensor("w", [P, NCH * J], F16, kind="ExternalInput")
    b_d = nc.dram_tensor("b", [P, J], F32, kind="ExternalInput")
    e_d = nc.dram_tensor("ident", [P, P], F16, kind="ExternalInput")
    y_d = nc.dram_tensor("y", [P, NT * 5], F32, kind="ExternalOutput")

    with tile.TileContext(nc) as tc:
        with (
            tc.tile_pool(name="const", bufs=1) as cpool,
            tc.tile_pool(name="xin", bufs=8) as xin_pool,
            tc.tile_pool(name="hi", bufs=8) as hi_pool,
            tc.tile_pool(name="xt", bufs=6) as xt_pool,
            tc.tile_pool(name="tp", bufs=6, space="PSUM") as tp_pool,
            tc.tile_pool(name="acc", bufs=2, space="PSUM") as acc_pool,
            tc.tile_pool(name="work", bufs=1) as wpool,
            tc.tile_pool(name="tailp", bufs=2) as tpool,
        ):
            # The identity leads the sync queue (one 0.66us dma_start before
            # the x stream) so the first transposes are never gated on it.
            # w/b ride the gpsimd SWDGE queue; both transfers interleave with
            # the young x stream and land ~10.5us, in time for the first
            # (two-group-lagged) accumulation matmuls and folds.
            id_sb = cpool.tile([P, P], F16)
            nc.sync.dma_start(id_sb[:], e_d.ap())
            w_sb = cpool.tile([P, NCH, J], F16)
            nc.gpsimd.dma_start(w_sb[:].rearrange("p c j -> p (c j)"),
                                w_d.ap())
            b_sb = cpool.tile([P, J], F32)
            nc.gpsimd.dma_start(b_sb[:], b_d.ap())

            # logits staging, one tile per half so each half's tail only
            # depends on its own 8 batch tiles
            NTH = NT // 2
            all_st = [wpool.tile([P, NTH, J], F32, tag=f"st{h}",
                                 name=f"all_st{h}")
                      for h in range(2)]

            def tail(h):
                st = all_st[h][:]
                e_all = tpool.tile([P, NTH, J], F32, tag="e_all")
                nc.scalar.activation(e_all[:], st,
                                     mybir.ActivationFunctionType.Exp)
                EG = e_all[:, :, 0:4]
                EP = e_all[:, :, 4:8]
                EA = e_all[:, :, 8:13]

                # elementwise products on the otherwise-idle gpsimd engine;
                # free-dim reductions + reciprocal stay on DVE (gpsimd only
                # reduces along partitions)
                tmp4 = tpool.tile([P, NTH, 4], F32, tag="tmp4")
                nc.gpsimd.tensor_mul(tmp4[:], EP, EG)
                tmp2 = tpool.tile([P, NTH, 2], F32, tag="tmp2")
                nc.gpsimd.tensor_mul(tmp2[:], e_all[:, :, 4:8:2],
                                     e_all[:, :, 1:4:2])
                tmp2b = tpool.tile([P, NTH, 2], F32, tag="tmp2b")
                nc.gpsimd.tensor_mul(tmp2b[:], e_all[:, :, 5:8:2],
                                     e_all[:, :, 0:3:2])

                sg = tpool.tile([P, NTH], F32, tag="sg")
                nc.vector.tensor_reduce(sg[:], EG, axis=AX.X, op=ADD)
                sp = tpool.tile([P, NTH], F32, tag="sp")
                nc.vector.tensor_reduce(sp[:], EP, axis=AX.X, op=ADD)
                u3 = tpool.tile([P, NTH, 3], F32, tag="u3")
                nc.vector.tensor_reduce(u3[:, :, 0], tmp4[:], axis=AX.X,
                                        op=ADD)
                nc.vector.tensor_reduce(u3[:, :, 1], tmp2[:], axis=AX.X,
                                        op=ADD)
                nc.vector.tensor_reduce(u3[:, :, 2], tmp2b[:], axis=AX.X,
                                        op=ADD)

                ss = tpool.tile([P, NTH], F32, tag="ss")
                nc.gpsimd.tensor_mul(ss[:], sp[:], sg[:])
                V = tpool.tile([P, NTH, 5], F32, tag="V")
                nc.gpsimd.tensor_sub(V[:, :, 0:3],
                                     ss[:].broadcast_to([P, NTH, 3]), u3[:])
                nc.gpsimd.tensor_copy(V[:, :, 3:5],
                                      ss[:].broadcast_to([P, NTH, 2]))
                tj = tpool.tile([P, NTH, 5], F32, tag="tj")
                nc.gpsimd.tensor_mul(tj[:], EA, V[:])

                s5 = tpool.tile([P, NTH], F32, tag="s5")
                nc.vector.tensor_reduce(s5[:], tj[:], axis=AX.X, op=ADD)
                r5 = tpool.tile([P, NTH], F32, tag="r5")
                nc.vector.reciprocal(r5[:], s5[:])

                out_sb = tpool.tile([P, NTH, 5], F32, tag="out_sb")
                nc.gpsimd.tensor_mul(out_sb[:], tj[:],
                                     r5[:].broadcast_to([P, NTH, 5]))
                # contiguous [128, 40] block (160B lines, 320B stride) on
                # the gpsimd software queue, in-order after the tail ops
                nc.gpsimd.dma_start(
                    y_d.ap()[:, h * NTH * 5:(h + 1) * NTH * 5],
                    out_sb[:].rearrange("p t j -> p (t j)"))

            def fold(t, acc):
                # PSUM->SBUF logits stage with the bias add fused, on DVE
                # (gpsimd cannot read PSUM; on ACT it would stall the
                # in-order convert stream whenever the PE lags)
                nc.vector.tensor_add(all_st[t // NTH][:, t % NTH, :],
                                     acc[:], b_sb[:])
                if t % NTH == NTH - 1:
                    tail(t // NTH)

            def emit_one(item):
                t, c, acc, xt, k = item
                nc.tensor.matmul(acc[:], xt[:, k * P:(k + 1) * P],
                                 w_sb[:, c, :],
                                 start=c == 0, stop=c == NCH - 1,
                                 skip_group_check=True)
                if c == NCH - 1:
                    fold(t, acc)

            # Chunk-level software pipelining: each accumulation matmul is
            # emitted interleaved between transposes, two GROUPS after its
            # own transpose, so its DVE copy (PSUM read + semaphore hops,
            # ~750ns latency) is long complete and the in-order PE stream
            # alternates T/M with no waits. A group-level lag gets clumped
            # into 8T+8M by the tile scheduler, which re-exposes the copy
            # latency ~2x per tile.
            pend = []
            LAGC = 8

            for t in range(NT):
                # half-tile transfers (4 KiB lines): best balance of HBM
                # burst efficiency and pipeline granularity measured. Tile 0
                # uses quarter tiles so the first convert/transpose chain
                # starts ~1.5us earlier during the DMA ramp.
                NP = 4 if t == 0 else 2
                CW_ = H // NP
                hq = []
                for q in range(NP):
                    xqt = xin_pool.tile([P, CW_], F32, tag=f"xh{NP}_{q}",
                                        name=f"xh{t}_{q}")
                    nc.sync.dma_start(
                        xqt[:],
                        x_d.ap()[t * P:(t + 1) * P,
                                 q * CW_:(q + 1) * CW_])
                    hqt = hi_pool.tile([P, CW_], F16, tag=f"hh{NP}_{q}",
                                       name=f"hh{t}_{q}")
                    nc.scalar.copy(hqt[:], xqt[:])  # fp16 round on ACT
                    hq.append(hqt)

                def chunk(c, hq=hq, cpp=CW_ // P):
                    return hq[c // cpp][:, (c % cpp) * P:(c % cpp + 1) * P]

                acc = acc_pool.tile([P, J], F32)
                for g in range(NG):
                    tp = tp_pool.tile([P, GC * P], F16)
                    for k in range(GC):
                        c = GC * g + k
                        nc.tensor.transpose(
                            tp[:, k * P:(k + 1) * P],
                            chunk(c),
                            id_sb[:])
                        if len(pend) > LAGC:
                            emit_one(pend.pop(0))
                    xt = xt_pool.tile([P, GC * P], F16, tag="xt")
                    nc.vector.tensor_copy(xt[:], tp[:])  # PSUM->SBUF
                    for k in range(GC):
                        pend.append((t, GC * g + k, acc, xt, k))
            for p in pend:
                emit_one(p)

    nc.compile()
    return nc


_NC_CACHE = {}


def _get_program(mode=MODE):
    if mode not in _NC_CACHE:
        _NC_CACHE[mode] = _build_program(mode)
    return _NC_CACHE[mode]


def _prep_in_maps(x, Wg, bg, Wp, bp, Wa, ba, mode=MODE):
    x = np.ascontiguousarray(np.asarray(x, dtype=np.float32))
    W = np.concatenate([np.asarray(Wg), np.asarray(Wp), np.asarray(Wa)],
                       axis=1).astype(np.float32)
    bvec = np.concatenate([np.asarray(bg), np.asarray(bp), np.asarray(ba)]
                          ).astype(np.float32).reshape(1, J)
    ident = np.eye(P, dtype=np.float16)
    # [h, j] -> [p, c*J+j] with h = c*128 + p (contiguous device load)
    w_dev = np.ascontiguousarray(
        W.astype(np.float16).reshape(NCH, P, J).transpose(1, 0, 2)
    ).reshape(P, NCH * J)
    b_dev = np.ascontiguousarray(np.broadcast_to(bvec, (P, J)),
                                 dtype=np.float32)
    in_maps = []
    for i in range(N_CORES):
        in_maps.append({
            "x": x[i * B:(i + 1) * B],
            "w": w_dev,
            "b": b_dev,
            "ident": ident,
        })
    return in_maps


def kernel(x, Wg, bg, Wp, bp, Wa, ba):
    in_maps = _prep_in_maps(x, Wg, bg, Wp, bp, Wa, ba)
    nc = _get_program()
    res = run_bass_kernel_spmd(nc, in_maps, core_ids=list(range(N_CORES)))
    outs = []
    for i in range(N_CORES):
        y = res.results[i]["y"]  # [P, NT*5], row b = t*P + p at [p, t*5+j]
        outs.append(y.reshape(P, NT, 5).transpose(1, 0, 2).reshape(B, 5))
    return np.concatenate(outs, axis=0)


# revision 21
# speedup vs baseline: 1.2980x; 1.2980x over previous
"""Trainium2 Bass kernel for DPL safe-policy head.

Computes, for x:[B,H] and three tiny heads Wg/Wp/Wa (4/4/5 logits):
    ghost  = softmax(x@Wg + bg); pacman = softmax(x@Wp + bp); base = softmax(x@Wa + ba)
    unsafe[b,a] = sum_cd pacman[b,c] * T[a,c,d] * ghost[b,d]   (T fixed 0/1 tensor)
    out = base*(1-unsafe) / sum(...)

Closed form used on device (softmax normalizations cancel except ghost/pacman's,
which fold into Sp*Sg):
    E = exp(logits), Sg = sum(EG), Sp = sum(EP), SS = Sp*Sg
    u0 = sum_c EPc*EGc ; u1 = EP0*EG1+EP2*EG3 ; u2 = EP1*EG0+EP3*EG2
    t_j = EA_j * (SS - u_j)  (u3 = u4 = 0);  out_j = t_j / sum_j t_j

Sharding: pure data parallel over batch across 8 cores (2048 rows each).

Per core pipeline (memory-bound target: stream x once from HBM at ~330GB/s;
PE issue rate is the co-bottleneck, so every engine has exactly one job):
  - x streams through the sync HWDGE queue as half-tile [128, 1024] DMAs
    (4 KiB lines); w (host-pretransposed, contiguous), bias and identity
    ride the scalar queue so they are resident in the first ~2us
  - ACT converts each half-tile to fp16 (one pass over x) + 2 exps
  - PE: per 128x128 chunk, one fp16 transpose (1 cy/row) and one fp16
    accumulation matmul (N=13); bias is folded in by the DVE instead of a
    rank-1 matmul; matmul emission lags transposes by one group so the
    in-order PE stream never waits on the DVE copy
  - DVE copies PSUM->SBUF fp16 transposed operands + per-tile bias-add fold
    (reads PSUM, which gpsimd cannot) + 2 reciprocals
  - GpSimd runs the whole logic-layer tail (SBUF-only tensor ops) and the
    output DMA on its software queue, so the DVE/ACT streams never pause
  - output written as one contiguous [128, NT*5] block per half (160B
    partition lines); host reorders to [B, 5]

fp16 single-term matmul (f16x1): max rel err ~1.5e-3 vs the fp32 reference
(harness gate 2e-2).
"""

import numpy as np

import concourse.bass as bass
import concourse.bacc as bacc
import concourse.mybir as mybir
import concourse.tile as tile
from concourse.bass_utils import run_bass_kernel_spmd

F32 = mybir.dt.float32
F16 = mybir.dt.float16
AX = mybir.AxisListType
ADD = mybir.AluOpType.add
SUB = mybir.AluOpType.subtract

MODE = "f16pre"

N_CORES = 8
B_FULL, H = 16384, 2048
B = B_FULL // N_CORES  # rows per core
P = 128
NT = B // P            # batch tiles per core
NCH = H // P           # contraction chunks
GC = 4                 # chunks per psum transpose group
NG = NCH // GC
J = 13                 # 4 + 4 + 5 logits


def _build_program(mode):
    assert mode == "f16pre"
    nc = bacc.Bacc("TRN2", target_bir_lowering=False, debug=False,
                   num_devices=N_CORES)
    x_d = nc.dram_tensor("x", [B, H], F32, kind="ExternalInput")
    w_d = nc.dram_tensor("w", [P, NCH * J], F16, kind="ExternalInput")
    b_d = nc.dram_tensor("b", [P, J], F32, kind="ExternalInput")
    e_d = nc.dram_tensor("ident", [P, P], F16, kind="ExternalInput")
    y_d = nc.dram_tensor("y", [P, NT * 5], F32, kind="ExternalOutput")

    with tile.TileContext(nc) as tc:
        with (
            tc.tile_pool(name="const", bufs=1) as cpool,
            tc.tile_pool(name="xin", bufs=8) as xin_pool,
            tc.tile_pool(name="hi", bufs=8) as hi_pool,
            tc.tile_pool(name="xt", bufs=6) as xt_pool,
            tc.tile_pool(name="tp", bufs=6, space="PSUM") as tp_pool,
            tc.tile_pool(name="acc", bufs=2, space="PSUM") as acc_pool,
            tc.tile_pool(name="work", bufs=1) as wpool,
            tc.tile_pool(name="tailp", bufs=2) as tpool,
        ):
            # The identity leads the sync queue (one 0.66us dma_start before
            # the x stream) so the first transposes are never gated on it.
            # w/b ride the gpsimd SWDGE queue; both transfers interleave with
            # the young x stream and land ~10.5us, in time for the first
            # (two-group-lagged) accumulation matmuls and folds.
            id_sb = cpool.tile([P, P], F16)
            nc.sync.dma_start(id_sb[:], e_d.ap())
            w_sb = cpool.tile([P, NCH, J], F16)
            nc.gpsimd.dma_start(w_sb[:].rearrange("p c j -> p (c j)"),
                                w_d.ap())
            b_sb = cpool.tile([P, J], F32)
            nc.gpsimd.dma_start(b_sb[:], b_d.ap())

            # logits staging, one tile per half so each half's tail only
            # depends on its own 8 batch tiles
            NTH = NT // 2
            all_st = [wpool.tile([P, NTH, J], F32, tag=f"st{h}",
                                 name=f"all_st{h}")
                      for h in range(2)]

            def tail(h):
                st = all_st[h][:]
                e_all = tpool.tile([P, NTH, J], F32, tag="e_all")
                nc.scalar.activation(e_all[:], st,
                                     mybir.ActivationFunctionType.Exp)
                EG = e_all[:, :, 0:4]
                EP = e_all[:, :, 4:8]
                EA = e_all[:, :, 8:13]

                # elementwise products on the otherwise-idle gpsimd engine;
                # free-dim reductions + reciprocal stay on DVE (gpsimd only
                # reduces along partitions)
                tmp4 = tpool.tile([P, NTH, 4], F32, tag="tmp4")
                nc.gpsimd.tensor_mul(tmp4[:], EP, EG)
                tmp2 = tpool.tile([P, NTH, 2], F32, tag="tmp2")
                nc.gpsimd.tensor_mul(tmp2[:], e_all[:, :, 4:8:2],
                                     e_all[:, :, 1:4:2])
                tmp2b = tpool.tile([P, NTH, 2], F32, tag="tmp2b")
                nc.gpsimd.tensor_mul(tmp2b[:], e_all[:, :, 5:8:2],
                                     e_all[:, :, 0:3:2])

                sg = tpool.tile([P, NTH], F32, tag="sg")
                nc.vector.tensor_reduce(sg[:], EG, axis=AX.X, op=ADD)
                sp = tpool.tile([P, NTH], F32, tag="sp")
                nc.vector.tensor_reduce(sp[:], EP, axis=AX.X, op=ADD)
                u3 = tpool.tile([P, NTH, 3], F32, tag="u3")
                nc.vector.tensor_reduce(u3[:, :, 0], tmp4[:], axis=AX.X,
                                        op=ADD)
                nc.vector.tensor_reduce(u3[:, :, 1], tmp2[:], axis=AX.X,
                                        op=ADD)
                nc.vector.tensor_reduce(u3[:, :, 2], tmp2b[:], axis=AX.X,
                                        op=ADD)

                ss = tpool.tile([P, NTH], F32, tag="ss")
                nc.gpsimd.tensor_mul(ss[:], sp[:], sg[:])
                V = tpool.tile([P, NTH, 5], F32, tag="V")
                nc.gpsimd.tensor_sub(V[:, :, 0:3],
                                     ss[:].broadcast_to([P, NTH, 3]), u3[:])
                nc.gpsimd.tensor_copy(V[:, :, 3:5],
                                      ss[:].broadcast_to([P, NTH, 2]))
                tj = tpool.tile([P, NTH, 5], F32, tag="tj")
                nc.gpsimd.tensor_mul(tj[:], EA, V[:])

                s5 = tpool.tile([P, NTH], F32, tag="s5")
                nc.vector.tensor_reduce(s5[:], tj[:], axis=AX.X, op=ADD)
                r5 = tpool.tile([P, NTH], F32, tag="r5")
                nc.vector.reciprocal(r5[:], s5[:])

                out_sb = tpool.tile([P, NTH, 5], F32, tag="out_sb")
                nc.gpsimd.tensor_mul(out_sb[:], tj[:],
                                     r5[:].broadcast_to([P, NTH, 5]))
                # contiguous [128, 40] block (160B lines, 320B stride) on
                # the gpsimd software queue, in-order after the tail ops
                nc.gpsimd.dma_start(
                    y_d.ap()[:, h * NTH * 5:(h + 1) * NTH * 5],
                    out_sb[:].rearrange("p t j -> p (t j)"))

            def fold(t, acc):
                # PSUM->SBUF logits stage with the bias add fused, on DVE
                # (gpsimd cannot read PSUM; on ACT it would stall the
                # in-order convert stream whenever the PE lags)
                nc.vector.tensor_add(all_st[t // NTH][:, t % NTH, :],
                                     acc[:], b_sb[:])
                if t % NTH == NTH - 1:
                    tail(t // NTH)

            def emit_one(item):
                t, c, acc, xt, k = item
                nc.tensor.matmul(acc[:], xt[:, k * P:(k + 1) * P],
                                 w_sb[:, c, :],
                                 start=c == 0, stop=c == NCH - 1,
                                 skip_group_check=True)
                if c == NCH - 1:
                    fold(t, acc)

            # Chunk-level software pipelining: each accumulation matmul is
            # emitted interleaved between transposes, two GROUPS after its
            # own transpose, so its DVE copy (PSUM read + semaphore hops,
            # ~750ns latency) is long complete and the in-order PE stream
            # alternates T/M with no waits. A group-level lag gets clumped
            # into 8T+8M by the tile scheduler, which re-exposes the copy
            # latency ~2x per tile.
            pend = []
            LAGC = 8

            for t in range(NT):
                # half-tile transfers (4 KiB lines): best balance of HBM
                # burst efficiency and pipeline granularity measured. Tile 0
                # uses quarter tiles so the first convert/transpose chain
                # starts ~1.5us earlier during the DMA ramp.
                NP = 4 if t == 0 else 2
                CW_ = H // NP
                hq = []
                for q in range(NP):
                    xqt = xin_pool.tile([P, CW_], F32, tag=f"xh{NP}_{q}",
                                        name=f"xh{t}_{q}")
                    nc.sync.dma_start(
                        xqt[:],
                        x_d.ap()[t * P:(t + 1) * P,
                                 q * CW_:(q + 1) * CW_])
                    hqt = hi_pool.tile([P, CW_], F16, tag=f"hh{NP}_{q}",
                                       name=f"hh{t}_{q}")
                    nc.scalar.copy(hqt[:], xqt[:])  # fp16 round on ACT
                    hq.append(hqt)

                def chunk(c, hq=hq, cpp=CW_ // P):
                    return hq[c // cpp][:, (c % cpp) * P:(c % cpp + 1) * P]

                acc = acc_pool.tile([P, J], F32)
                for g in range(NG):
                    tp = tp_pool.tile([P, GC * P], F16)
                    xt = xt_pool.tile([P, GC * P], F16, tag="xt")
                    for k in range(GC):
                        c = GC * g + k
                        nc.tensor.transpose(
                            tp[:, k * P:(k + 1) * P],
                            chunk(c),
                            id_sb[:])
                        if len(pend) > LAGC:
                            emit_one(pend.pop(0))
                        if k % 2 == 1:
                            # PSUM->SBUF in chunk pairs: each pair's copy
                            # completes during the next transposes, so the
                            # scheduler's clumped 8T+8M order never exposes
                            # the copy latency
                            nc.vector.tensor_copy(
                                xt[:, (k - 1) * P:(k + 1) * P],
                                tp[:, (k - 1) * P:(k + 1) * P])
                    for k in range(GC):
                        pend.append((t, GC * g + k, acc, xt, k))
            for p in pend:
                emit_one(p)

    nc.compile()
    return nc


_NC_CACHE = {}


def _get_program(mode=MODE):
    if mode not in _NC_CACHE:
        _NC_CACHE[mode] = _build_program(mode)
    return _NC_CACHE[mode]


def _prep_in_maps(x, Wg, bg, Wp, bp, Wa, ba, mode=MODE):
    x = np.ascontiguousarray(np.asarray(x, dtype=np.float32))
    W = np.concatenate([np.asarray(Wg), np.asarray(Wp), np.asarray(Wa)],
                       axis=1).astype(np.float32)
    bvec = np.concatenate([np.asarray(bg), np.asarray(bp), np.asarray(ba)]
                          ).astype(np.float32).reshape(1, J)
    ident = np.eye(P, dtype=np.float16)
    # [h, j] -> [p, c*J+j] with h = c*128 + p (contiguous device load)
    w_dev = np.ascontiguousarray(
        W.astype(np.float16).reshape(NCH, P, J).transpose(1, 0, 2)
    ).reshape(P, NCH * J)
    b_dev = np.ascontiguousarray(np.broadcast_to(bvec, (P, J)),
                                 dtype=np.float32)
    in_maps = []
    for i in range(N_CORES):
        in_maps.append({
            "x": x[i * B:(i + 1) * B],
            "w": w_dev,
            "b": b_dev,
            "ident": ident,
        })
    return in_maps


def kernel(x, Wg, bg, Wp, bp, Wa, ba):
    in_maps = _prep_in_maps(x, Wg, bg, Wp, bp, Wa, ba)
    nc = _get_program()
    res = run_bass_kernel_spmd(nc, in_maps, core_ids=list(range(N_CORES)))
    outs = []
    for i in range(N_CORES):
        y = res.results[i]["y"]  # [P, NT*5], row b = t*P + p at [p, t*5+j]
        outs.append(y.reshape(P, NT, 5).transpose(1, 0, 2).reshape(B, 5))
    return np.concatenate(outs, axis=0)


# revision 23
# speedup vs baseline: 1.3250x; 1.0208x over previous
"""Trainium2 Bass kernel for DPL safe-policy head.

Computes, for x:[B,H] and three tiny heads Wg/Wp/Wa (4/4/5 logits):
    ghost  = softmax(x@Wg + bg); pacman = softmax(x@Wp + bp); base = softmax(x@Wa + ba)
    unsafe[b,a] = sum_cd pacman[b,c] * T[a,c,d] * ghost[b,d]   (T fixed 0/1 tensor)
    out = base*(1-unsafe) / sum(...)

Closed form used on device (softmax normalizations cancel except ghost/pacman's,
which fold into Sp*Sg):
    E = exp(logits), Sg = sum(EG), Sp = sum(EP), SS = Sp*Sg
    u0 = sum_c EPc*EGc ; u1 = EP0*EG1+EP2*EG3 ; u2 = EP1*EG0+EP3*EG2
    t_j = EA_j * (SS - u_j)  (u3 = u4 = 0);  out_j = t_j / sum_j t_j

Sharding: pure data parallel over batch across 8 cores (2048 rows each).

Per core pipeline (memory-bound target: stream x once from HBM at ~330GB/s;
PE issue rate is the co-bottleneck, so every engine has exactly one job):
  - x streams through the sync HWDGE queue as half-tile [128, 1024] DMAs
    (4 KiB lines); w (host-pretransposed, contiguous), bias and identity
    ride the scalar queue so they are resident in the first ~2us
  - ACT converts each half-tile to fp16 (one pass over x) + 2 exps
  - PE: per 128x128 chunk, one fp16 transpose (1 cy/row) and one fp16
    accumulation matmul (N=13); bias is folded in by the DVE instead of a
    rank-1 matmul; matmul emission lags transposes by one group so the
    in-order PE stream never waits on the DVE copy
  - DVE copies PSUM->SBUF fp16 transposed operands + per-tile bias-add fold
    (reads PSUM, which gpsimd cannot) + 2 reciprocals
  - GpSimd runs the whole logic-layer tail (SBUF-only tensor ops) and the
    output DMA on its software queue, so the DVE/ACT streams never pause
  - output written as one contiguous [128, NT*5] block per half (160B
    partition lines); host reorders to [B, 5]

fp16 single-term matmul (f16x1): max rel err ~1.5e-3 vs the fp32 reference
(harness gate 2e-2).
"""

import numpy as np

import concourse.bass as bass
import concourse.bacc as bacc
import concourse.mybir as mybir
import concourse.tile as tile
from concourse.bass_utils import run_bass_kernel_spmd

F32 = mybir.dt.float32
F16 = mybir.dt.float16
AX = mybir.AxisListType
ADD = mybir.AluOpType.add
SUB = mybir.AluOpType.subtract

MODE = "f16pre"

N_CORES = 8
B_FULL, H = 16384, 2048
B = B_FULL // N_CORES  # rows per core
P = 128
NT = B // P            # batch tiles per core
NCH = H // P           # contraction chunks
GC = 4                 # chunks per psum transpose group
NG = NCH // GC
J = 13                 # 4 + 4 + 5 logits


def _build_program(mode):
    assert mode == "f16pre"
    nc = bacc.Bacc("TRN2", target_bir_lowering=False, debug=False,
                   num_devices=N_CORES)
    x_d = nc.dram_tensor("x", [B, H], F32, kind="ExternalInput")
    w_d = nc.dram_tensor("w", [P, NCH * J], F16, kind="ExternalInput")
    b_d = nc.dram_tensor("b", [P, J], F32, kind="ExternalInput")
    e_d = nc.dram_tensor("ident", [P, P], F16, kind="ExternalInput")
    y_d = nc.dram_tensor("y", [P, NT * 5], F32, kind="ExternalOutput")

    with tile.TileContext(nc) as tc:
        with (
            tc.tile_pool(name="const", bufs=1) as cpool,
            tc.tile_pool(name="xin", bufs=8) as xin_pool,
            tc.tile_pool(name="hi", bufs=8) as hi_pool,
            tc.tile_pool(name="xt", bufs=6) as xt_pool,
            tc.tile_pool(name="tp", bufs=6, space="PSUM") as tp_pool,
            tc.tile_pool(name="acc", bufs=2, space="PSUM") as acc_pool,
            tc.tile_pool(name="work", bufs=1) as wpool,
            tc.tile_pool(name="tailp", bufs=2) as tpool,
        ):
            # The identity leads the sync queue (one 0.66us dma_start before
            # the x stream) so the first transposes are never gated on it.
            # w/b ride the gpsimd SWDGE queue; both transfers interleave with
            # the young x stream and land ~10.5us, in time for the first
            # (two-group-lagged) accumulation matmuls and folds.
            id_sb = cpool.tile([P, P], F16)
            nc.sync.dma_start(id_sb[:], e_d.ap())
            w_sb = cpool.tile([P, NCH, J], F16)
            nc.gpsimd.dma_start(w_sb[:].rearrange("p c j -> p (c j)"),
                                w_d.ap())
            b_sb = cpool.tile([P, J], F32)
            nc.gpsimd.dma_start(b_sb[:], b_d.ap())

            # logits staging, one tile per half so each half's tail only
            # depends on its own 8 batch tiles
            NTH = NT // 2
            all_st = [wpool.tile([P, NTH, J], F32, tag=f"st{h}",
                                 name=f"all_st{h}")
                      for h in range(2)]

            def tail(h):
                st = all_st[h][:]
                e_all = tpool.tile([P, NTH, J], F32, tag="e_all")
                nc.scalar.activation(e_all[:], st,
                                     mybir.ActivationFunctionType.Exp)
                EG = e_all[:, :, 0:4]
                EP = e_all[:, :, 4:8]
                EA = e_all[:, :, 8:13]

                # elementwise products on the otherwise-idle gpsimd engine;
                # free-dim reductions + reciprocal stay on DVE (gpsimd only
                # reduces along partitions)
                tmp4 = tpool.tile([P, NTH, 4], F32, tag="tmp4")
                nc.gpsimd.tensor_mul(tmp4[:], EP, EG)
                tmp2 = tpool.tile([P, NTH, 2], F32, tag="tmp2")
                nc.gpsimd.tensor_mul(tmp2[:], e_all[:, :, 4:8:2],
                                     e_all[:, :, 1:4:2])
                tmp2b = tpool.tile([P, NTH, 2], F32, tag="tmp2b")
                nc.gpsimd.tensor_mul(tmp2b[:], e_all[:, :, 5:8:2],
                                     e_all[:, :, 0:3:2])

                sg = tpool.tile([P, NTH], F32, tag="sg")
                nc.vector.tensor_reduce(sg[:], EG, axis=AX.X, op=ADD)
                sp = tpool.tile([P, NTH], F32, tag="sp")
                nc.vector.tensor_reduce(sp[:], EP, axis=AX.X, op=ADD)
                u3 = tpool.tile([P, NTH, 3], F32, tag="u3")
                nc.vector.tensor_reduce(u3[:, :, 0], tmp4[:], axis=AX.X,
                                        op=ADD)
                nc.vector.tensor_reduce(u3[:, :, 1], tmp2[:], axis=AX.X,
                                        op=ADD)
                nc.vector.tensor_reduce(u3[:, :, 2], tmp2b[:], axis=AX.X,
                                        op=ADD)

                ss = tpool.tile([P, NTH], F32, tag="ss")
                nc.gpsimd.tensor_mul(ss[:], sp[:], sg[:])
                V = tpool.tile([P, NTH, 5], F32, tag="V")
                nc.gpsimd.tensor_sub(V[:, :, 0:3],
                                     ss[:].broadcast_to([P, NTH, 3]), u3[:])
                nc.gpsimd.tensor_copy(V[:, :, 3:5],
                                      ss[:].broadcast_to([P, NTH, 2]))
                tj = tpool.tile([P, NTH, 5], F32, tag="tj")
                nc.gpsimd.tensor_mul(tj[:], EA, V[:])

                s5 = tpool.tile([P, NTH], F32, tag="s5")
                nc.vector.tensor_reduce(s5[:], tj[:], axis=AX.X, op=ADD)
                r5 = tpool.tile([P, NTH], F32, tag="r5")
                nc.vector.reciprocal(r5[:], s5[:])

                out_sb = tpool.tile([P, NTH, 5], F32, tag="out_sb")
                nc.gpsimd.tensor_mul(out_sb[:], tj[:],
                                     r5[:].broadcast_to([P, NTH, 5]))
                # contiguous [128, 40] block (160B lines, 320B stride) on
                # the gpsimd software queue, in-order after the tail ops
                nc.gpsimd.dma_start(
                    y_d.ap()[:, h * NTH * 5:(h + 1) * NTH * 5],
                    out_sb[:].rearrange("p t j -> p (t j)"))

            def fold(t, acc):
                # PSUM->SBUF logits stage with the bias add fused, on DVE
                # (gpsimd cannot read PSUM; on ACT it would stall the
                # in-order convert stream whenever the PE lags)
                nc.vector.tensor_add(all_st[t // NTH][:, t % NTH, :],
                                     acc[:], b_sb[:])
                if t % NTH == NTH - 1:
                    tail(t // NTH)

            def emit_one(item):
                t, c, acc, xt, k = item
                nc.tensor.matmul(acc[:], xt[:, k * P:(k + 1) * P],
                                 w_sb[:, c, :],
                                 start=c == 0, stop=c == NCH - 1,
                                 skip_group_check=True)
                if c == NCH - 1:
                    fold(t, acc)

            # Chunk-level software pipelining: each accumulation matmul is
            # emitted interleaved between transposes, two GROUPS after its
            # own transpose, so its DVE copy (PSUM read + semaphore hops,
            # ~750ns latency) is long complete and the in-order PE stream
            # alternates T/M with no waits. A group-level lag gets clumped
            # into 8T+8M by the tile scheduler, which re-exposes the copy
            # latency ~2x per tile.
            pend = []
            LAGC = 12

            for t in range(NT):
                # half-tile transfers (4 KiB lines): best balance of HBM
                # burst efficiency and pipeline granularity measured. Tile 0
                # uses quarter tiles so the first convert/transpose chain
                # starts ~1.5us earlier during the DMA ramp.
                NP = 4 if t == 0 else 2
                CW_ = H // NP
                hq = []
                for q in range(NP):
                    xqt = xin_pool.tile([P, CW_], F32, tag=f"xh{NP}_{q}",
                                        name=f"xh{t}_{q}")
                    nc.sync.dma_start(
                        xqt[:],
                        x_d.ap()[t * P:(t + 1) * P,
                                 q * CW_:(q + 1) * CW_])
                    hqt = hi_pool.tile([P, CW_], F16, tag=f"hh{NP}_{q}",
                                       name=f"hh{t}_{q}")
                    nc.scalar.copy(hqt[:], xqt[:])  # fp16 round on ACT
                    hq.append(hqt)

                def chunk(c, hq=hq, cpp=CW_ // P):
                    return hq[c // cpp][:, (c % cpp) * P:(c % cpp + 1) * P]

                acc = acc_pool.tile([P, J], F32)
                for g in range(NG):
                    tp = tp_pool.tile([P, GC * P], F16)
                    for k in range(GC):
                        c = GC * g + k
                        nc.tensor.transpose(
                            tp[:, k * P:(k + 1) * P],
                            chunk(c),
                            id_sb[:])
                        if len(pend) > LAGC:
                            emit_one(pend.pop(0))
                    xt = xt_pool.tile([P, GC * P], F16, tag="xt")
                    nc.vector.tensor_copy(xt[:], tp[:])  # PSUM->SBUF
                    for k in range(GC):
                        pend.append((t, GC * g + k, acc, xt, k))
            for p in pend:
                emit_one(p)

    nc.compile()
    return nc


_NC_CACHE = {}


def _get_program(mode=MODE):
    if mode not in _NC_CACHE:
        _NC_CACHE[mode] = _build_program(mode)
    return _NC_CACHE[mode]


def _prep_in_maps(x, Wg, bg, Wp, bp, Wa, ba, mode=MODE):
    x = np.ascontiguousarray(np.asarray(x, dtype=np.float32))
    W = np.concatenate([np.asarray(Wg), np.asarray(Wp), np.asarray(Wa)],
                       axis=1).astype(np.float32)
    bvec = np.concatenate([np.asarray(bg), np.asarray(bp), np.asarray(ba)]
                          ).astype(np.float32).reshape(1, J)
    ident = np.eye(P, dtype=np.float16)
    # [h, j] -> [p, c*J+j] with h = c*128 + p (contiguous device load)
    w_dev = np.ascontiguousarray(
        W.astype(np.float16).reshape(NCH, P, J).transpose(1, 0, 2)
    ).reshape(P, NCH * J)
    b_dev = np.ascontiguousarray(np.broadcast_to(bvec, (P, J)),
                                 dtype=np.float32)
    in_maps = []
    for i in range(N_CORES):
        in_maps.append({
            "x": x[i * B:(i + 1) * B],
            "w": w_dev,
            "b": b_dev,
            "ident": ident,
        })
    return in_maps


def kernel(x, Wg, bg, Wp, bp, Wa, ba):
    in_maps = _prep_in_maps(x, Wg, bg, Wp, bp, Wa, ba)
    nc = _get_program()
    res = run_bass_kernel_spmd(nc, in_maps, core_ids=list(range(N_CORES)))
    outs = []
    for i in range(N_CORES):
        y = res.results[i]["y"]  # [P, NT*5], row b = t*P + p at [p, t*5+j]
        outs.append(y.reshape(P, NT, 5).transpose(1, 0, 2).reshape(B, 5))
    return np.concatenate(outs, axis=0)


# revision 24
# speedup vs baseline: 1.4686x; 1.1084x over previous
"""Trainium2 Bass kernel for DPL safe-policy head.

Computes, for x:[B,H] and three tiny heads Wg/Wp/Wa (4/4/5 logits):
    ghost  = softmax(x@Wg + bg); pacman = softmax(x@Wp + bp); base = softmax(x@Wa + ba)
    unsafe[b,a] = sum_cd pacman[b,c] * T[a,c,d] * ghost[b,d]   (T fixed 0/1 tensor)
    out = base*(1-unsafe) / sum(...)

Closed form used on device (softmax normalizations cancel except ghost/pacman's,
which fold into Sp*Sg):
    E = exp(logits), Sg = sum(EG), Sp = sum(EP), SS = Sp*Sg
    u0 = sum_c EPc*EGc ; u1 = EP0*EG1+EP2*EG3 ; u2 = EP1*EG0+EP3*EG2
    t_j = EA_j * (SS - u_j)  (u3 = u4 = 0);  out_j = t_j / sum_j t_j

Sharding: pure data parallel over batch across 8 cores (2048 rows each).

Per core pipeline (memory-bound target: stream x once from HBM at ~330GB/s;
PE issue rate is the co-bottleneck, so every engine has exactly one job):
  - x streams through the sync HWDGE queue as half-tile [128, 1024] DMAs
    (4 KiB lines); w (host-pretransposed, contiguous), bias and identity
    ride the scalar queue so they are resident in the first ~2us
  - ACT converts each half-tile to fp16 (one pass over x) + 2 exps
  - PE: per 128x128 chunk, one fp16 transpose (1 cy/row) and one fp16
    accumulation matmul (N=13); bias is folded in by the DVE instead of a
    rank-1 matmul; matmul emission lags transposes by one group so the
    in-order PE stream never waits on the DVE copy
  - DVE copies PSUM->SBUF fp16 transposed operands + per-tile bias-add fold
    (reads PSUM, which gpsimd cannot) + 2 reciprocals
  - GpSimd runs the whole logic-layer tail (SBUF-only tensor ops) and the
    output DMA on its software queue, so the DVE/ACT streams never pause
  - output written as one contiguous [128, NT*5] block per half (160B
    partition lines); host reorders to [B, 5]

fp16 single-term matmul (f16x1): max rel err ~1.5e-3 vs the fp32 reference
(harness gate 2e-2).
"""

import numpy as np

import concourse.bass as bass
import concourse.bacc as bacc
import concourse.mybir as mybir
import concourse.tile as tile
from concourse.bass_utils import run_bass_kernel_spmd

F32 = mybir.dt.float32
F16 = mybir.dt.float16
AX = mybir.AxisListType
ADD = mybir.AluOpType.add
SUB = mybir.AluOpType.subtract

MODE = "f16pre"

N_CORES = 8
B_FULL, H = 16384, 2048
B = B_FULL // N_CORES  # rows per core
P = 128
NT = B // P            # batch tiles per core
NCH = H // P           # contraction chunks
GC = 4                 # chunks per psum transpose group
NG = NCH // GC
J = 13                 # 4 + 4 + 5 logits


def _build_program(mode):
    assert mode == "f16pre"
    nc = bacc.Bacc("TRN2", target_bir_lowering=False, debug=False,
                   num_devices=N_CORES)
    x_d = nc.dram_tensor("x", [B, H], F32, kind="ExternalInput")
    w_d = nc.dram_tensor("w", [P, NCH * J], F16, kind="ExternalInput")
    b_d = nc.dram_tensor("b", [P, J], F32, kind="ExternalInput")
    e_d = nc.dram_tensor("ident", [P, P], F16, kind="ExternalInput")
    y_d = nc.dram_tensor("y", [P, NT * 5], F32, kind="ExternalOutput")

    with tile.TileContext(nc) as tc:
        with (
            tc.tile_pool(name="const", bufs=1) as cpool,
            tc.tile_pool(name="xin", bufs=8) as xin_pool,
            tc.tile_pool(name="hi", bufs=8) as hi_pool,
            tc.tile_pool(name="xt", bufs=6) as xt_pool,
            tc.tile_pool(name="tp", bufs=6, space="PSUM") as tp_pool,
            tc.tile_pool(name="acc", bufs=2, space="PSUM") as acc_pool,
            tc.tile_pool(name="work", bufs=1) as wpool,
            tc.tile_pool(name="tailp", bufs=2) as tpool,
        ):
            # The identity leads the sync queue (one 0.66us dma_start before
            # the x stream) so the first transposes are never gated on it.
            # w/b ride the gpsimd SWDGE queue; both transfers interleave with
            # the young x stream and land ~10.5us, in time for the first
            # (two-group-lagged) accumulation matmuls and folds.
            id_sb = cpool.tile([P, P], F16)
            nc.sync.dma_start(id_sb[:], e_d.ap())
            w_sb = cpool.tile([P, NCH, J], F16)
            nc.gpsimd.dma_start(w_sb[:].rearrange("p c j -> p (c j)"),
                                w_d.ap())
            b_sb = cpool.tile([P, J], F32)
            nc.gpsimd.dma_start(b_sb[:], b_d.ap())

            # logits staging, one tile per half so each half's tail only
            # depends on its own 8 batch tiles
            NTH = NT // 2
            all_st = [wpool.tile([P, NTH, J], F32, tag=f"st{h}",
                                 name=f"all_st{h}")
                      for h in range(2)]

            def tail(h):
                st = all_st[h][:]
                e_all = tpool.tile([P, NTH, J], F32, tag="e_all")
                nc.scalar.activation(e_all[:], st,
                                     mybir.ActivationFunctionType.Exp)
                EG = e_all[:, :, 0:4]
                EP = e_all[:, :, 4:8]
                EA = e_all[:, :, 8:13]

                # elementwise products on the otherwise-idle gpsimd engine;
                # free-dim reductions + reciprocal stay on DVE (gpsimd only
                # reduces along partitions)
                tmp4 = tpool.tile([P, NTH, 4], F32, tag="tmp4")
                nc.gpsimd.tensor_mul(tmp4[:], EP, EG)
                tmp2 = tpool.tile([P, NTH, 2], F32, tag="tmp2")
                nc.gpsimd.tensor_mul(tmp2[:], e_all[:, :, 4:8:2],
                                     e_all[:, :, 1:4:2])
                tmp2b = tpool.tile([P, NTH, 2], F32, tag="tmp2b")
                nc.gpsimd.tensor_mul(tmp2b[:], e_all[:, :, 5:8:2],
                                     e_all[:, :, 0:3:2])

                sg = tpool.tile([P, NTH], F32, tag="sg")
                nc.vector.tensor_reduce(sg[:], EG, axis=AX.X, op=ADD)
                sp = tpool.tile([P, NTH], F32, tag="sp")
                nc.vector.tensor_reduce(sp[:], EP, axis=AX.X, op=ADD)
                u3 = tpool.tile([P, NTH, 3], F32, tag="u3")
                nc.vector.tensor_reduce(u3[:, :, 0], tmp4[:], axis=AX.X,
                                        op=ADD)
                nc.vector.tensor_reduce(u3[:, :, 1], tmp2[:], axis=AX.X,
                                        op=ADD)
                nc.vector.tensor_reduce(u3[:, :, 2], tmp2b[:], axis=AX.X,
                                        op=ADD)

                ss = tpool.tile([P, NTH], F32, tag="ss")
                nc.gpsimd.tensor_mul(ss[:], sp[:], sg[:])
                V = tpool.tile([P, NTH, 5], F32, tag="V")
                nc.gpsimd.tensor_sub(V[:, :, 0:3],
                                     ss[:].broadcast_to([P, NTH, 3]), u3[:])
                nc.gpsimd.tensor_copy(V[:, :, 3:5],
                                      ss[:].broadcast_to([P, NTH, 2]))
                tj = tpool.tile([P, NTH, 5], F32, tag="tj")
                nc.gpsimd.tensor_mul(tj[:], EA, V[:])

                s5 = tpool.tile([P, NTH], F32, tag="s5")
                nc.vector.tensor_reduce(s5[:], tj[:], axis=AX.X, op=ADD)
                r5 = tpool.tile([P, NTH], F32, tag="r5")
                nc.vector.reciprocal(r5[:], s5[:])

                out_sb = tpool.tile([P, NTH, 5], F32, tag="out_sb")
                nc.gpsimd.tensor_mul(out_sb[:], tj[:],
                                     r5[:].broadcast_to([P, NTH, 5]))
                # contiguous [128, 40] block (160B lines, 320B stride) on
                # the gpsimd software queue, in-order after the tail ops
                nc.gpsimd.dma_start(
                    y_d.ap()[:, h * NTH * 5:(h + 1) * NTH * 5],
                    out_sb[:].rearrange("p t j -> p (t j)"))

            def fold(t, acc):
                # PSUM->SBUF logits stage with the bias add fused, on DVE
                # (gpsimd cannot read PSUM; on ACT it would stall the
                # in-order convert stream whenever the PE lags)
                nc.vector.tensor_add(all_st[t // NTH][:, t % NTH, :],
                                     acc[:], b_sb[:])
                if t % NTH == NTH - 1:
                    tail(t // NTH)

            def emit_one(item):
                t, c, acc, xt, k = item
                nc.tensor.matmul(acc[:], xt[:, k * P:(k + 1) * P],
                                 w_sb[:, c, :],
                                 start=c == 0, stop=c == NCH - 1,
                                 skip_group_check=True)
                if c == NCH - 1:
                    fold(t, acc)

            # Chunk-level software pipelining: each accumulation matmul is
            # emitted interleaved between transposes, two GROUPS after its
            # own transpose, so its DVE copy (PSUM read + semaphore hops,
            # ~750ns latency) is long complete and the in-order PE stream
            # alternates T/M with no waits. A group-level lag gets clumped
            # into 8T+8M by the tile scheduler, which re-exposes the copy
            # latency ~2x per tile.
            pend = []
            LAGC = 12

            for t in range(NT):
                # full-tile transfers (8 KiB lines) for HBM burst efficiency;
                # tile 0 uses quarter tiles so the first convert/transpose
                # chain starts ~1.5us earlier during the DMA ramp.
                NP = 4 if t == 0 else 1
                CW_ = H // NP
                hq = []
                for q in range(NP):
                    xqt = xin_pool.tile([P, CW_], F32, tag=f"xh{NP}_{q}",
                                        name=f"xh{t}_{q}")
                    nc.sync.dma_start(
                        xqt[:],
                        x_d.ap()[t * P:(t + 1) * P,
                                 q * CW_:(q + 1) * CW_])
                    hqt = hi_pool.tile([P, CW_], F16, tag=f"hh{NP}_{q}",
                                       name=f"hh{t}_{q}")
                    nc.scalar.copy(hqt[:], xqt[:])  # fp16 round on ACT
                    hq.append(hqt)

                def chunk(c, hq=hq, cpp=CW_ // P):
                    return hq[c // cpp][:, (c % cpp) * P:(c % cpp + 1) * P]

                acc = acc_pool.tile([P, J], F32)
                for g in range(NG):
                    tp = tp_pool.tile([P, GC * P], F16)
                    for k in range(GC):
                        c = GC * g + k
                        nc.tensor.transpose(
                            tp[:, k * P:(k + 1) * P],
                            chunk(c),
                            id_sb[:])
                        if len(pend) > LAGC:
                            emit_one(pend.pop(0))
                    xt = xt_pool.tile([P, GC * P], F16, tag="xt")
                    nc.vector.tensor_copy(xt[:], tp[:])  # PSUM->SBUF
                    for k in range(GC):
                        pend.append((t, GC * g + k, acc, xt, k))
            for p in pend:
                emit_one(p)

    nc.compile()
    return nc


_NC_CACHE = {}


def _get_program(mode=MODE):
    if mode not in _NC_CACHE:
        _NC_CACHE[mode] = _build_program(mode)
    return _NC_CACHE[mode]


def _prep_in_maps(x, Wg, bg, Wp, bp, Wa, ba, mode=MODE):
    x = np.ascontiguousarray(np.asarray(x, dtype=np.float32))
    W = np.concatenate([np.asarray(Wg), np.asarray(Wp), np.asarray(Wa)],
                       axis=1).astype(np.float32)
    bvec = np.concatenate([np.asarray(bg), np.asarray(bp), np.asarray(ba)]
                          ).astype(np.float32).reshape(1, J)
    ident = np.eye(P, dtype=np.float16)
    # [h, j] -> [p, c*J+j] with h = c*128 + p (contiguous device load)
    w_dev = np.ascontiguousarray(
        W.astype(np.float16).reshape(NCH, P, J).transpose(1, 0, 2)
    ).reshape(P, NCH * J)
    b_dev = np.ascontiguousarray(np.broadcast_to(bvec, (P, J)),
                                 dtype=np.float32)
    in_maps = []
    for i in range(N_CORES):
        in_maps.append({
            "x": x[i * B:(i + 1) * B],
            "w": w_dev,
            "b": b_dev,
            "ident": ident,
        })
    return in_maps


def kernel(x, Wg, bg, Wp, bp, Wa, ba):
    in_maps = _prep_in_maps(x, Wg, bg, Wp, bp, Wa, ba)
    nc = _get_program()
    res = run_bass_kernel_spmd(nc, in_maps, core_ids=list(range(N_CORES)))
    outs = []
    for i in range(N_CORES):
        y = res.results[i]["y"]  # [P, NT*5], row b = t*P + p at [p, t*5+j]
        outs.append(y.reshape(P, NT, 5).transpose(1, 0, 2).reshape(B, 5))
    return np.concatenate(outs, axis=0)
